# revision 9
# baseline (speedup 1.0000x reference)
"""Trainium2 Bass kernel for nn_Attention_44564580663760.

Single-head "attention" (B=8, S=2048, D=1024, fp32) with the reference's
quirk reproduced: scores = q @ v^T (k projection unused), causal mask,
softmax, ctx @ v, output projection.

Sharding: data-parallel — one batch element per NeuronCore (8 cores).

Per-core dataflow (all layouts chosen so no on-chip transpose of the
attention matrix is ever needed; matmul contracts the partition dim):
    xT  [d, s]   (host-transposed input)
    qT  [e, s] = WqT.T @ xT (+bq)         -> DRAM scratch, reloaded per block
    vT  [e, s] = WvT.T @ xT (+bv)         -> resident
    v   [k, d] = PE-transpose(vT) (bf16)  -> resident
    per q-block (512 cols):
      scoresT [k, q] = vT.T @ qT          (fp32r matmuls)
      eT = exp(scoresT/32) (bf16), causal-masked
      ctxT [d, q] = v.T-as-lhsT @ eT      (bf16 matmuls)
      l[q] = column sums of eT via N=1 matmuls with ones
      out [q, e] = ctxT-as-lhsT @ WoT, scaled by 1/l, + bo
"""

import sys

sys.path.insert(0, "/opt/trn_rl_repo")

import numpy as np

import concourse.bass as bass
import concourse.bacc as bacc
import concourse.mybir as mybir
import concourse.tile as tile
from concourse.bass_utils import run_bass_kernel_spmd
from concourse.masks import make_identity

FP32 = mybir.dt.float32
FP32R = mybir.dt.float32r
BF16 = mybir.dt.bfloat16

B, S, D = 8, 2048, 1024
PT = 128
NTS = S // PT  # 16 s-tiles
NTD = D // PT  # 8 d-tiles
QB = 512  # q-block width
NBLK = S // QB  # 4
NEC = D // 512  # 2 out-proj column chunks
SCALE = 1.0 / np.sqrt(np.float32(D))
ACT = mybir.ActivationFunctionType


def build_nc(causal: bool) -> bass.Bass:
    nc = bacc.Bacc("TRN2", target_bir_lowering=False, debug=False)
    xT_d = nc.declare_dram_parameter("xT", [D, S], BF16, isOutput=False)
    WqT_d = nc.declare_dram_parameter("WqT", [D, D], BF16, isOutput=False)
    WvT_d = nc.declare_dram_parameter("WvT", [D, D], BF16, isOutput=False)
    WoT_d = nc.declare_dram_parameter("WoT", [D, D], BF16, isOutput=False)
    bq_d = nc.declare_dram_parameter("bqc", [D, 1], FP32, isOutput=False)
    bv_d = nc.declare_dram_parameter("bvc", [D, 1], FP32, isOutput=False)
    bo_d = nc.declare_dram_parameter("bob", [PT, D], FP32, isOutput=False)
    ones_d = nc.declare_dram_parameter("ones_col", [PT, 1], BF16, isOutput=False)
    utri_d = nc.declare_dram_parameter("utri", [PT, PT], BF16, isOutput=False)
    out_d = nc.declare_dram_parameter("out", [S, D], FP32, isOutput=True)

    with (
        tile.TileContext(nc) as tc,
        tc.tile_pool(name="const", bufs=1) as constp,
        tc.tile_pool(name="dram", bufs=1, space="DRAM") as dramp,
        tc.tile_pool(name="vTp", bufs=1) as vTp,
    ):
        ident = constp.tile([PT, PT], BF16, tag="ident")
        make_identity(nc, ident[:])
        WoT_t = []
        for d in range(NTD):
            w = constp.tile([PT, D], BF16, tag=f"wot{d}", name=f"wot{d}")
            nc.sync.dma_start(w[:], WoT_d[d * PT : (d + 1) * PT, :])
            WoT_t.append(w)
        bo_t = constp.tile([PT, D], FP32, tag="bo")
        nc.sync.dma_start(bo_t[:], bo_d[:, :])
        ones_t = constp.tile([PT, 1], BF16, tag="ones")
        nc.sync.dma_start(ones_t[:], ones_d[:, :])
        utri_t = constp.tile([PT, PT], BF16, tag="utri")
        nc.sync.dma_start(utri_t[:], utri_d[:, :])
        bq_t, bv_t = [], []
        for e in range(NTD):
            tq = constp.tile([PT, 1], FP32, tag=f"bq{e}", name=f"bq{e}")
            nc.sync.dma_start(tq[:], bq_d[e * PT : (e + 1) * PT, :])
            bq_t.append(tq)
            tv = constp.tile([PT, 1], FP32, tag=f"bv{e}", name=f"bv{e}")
            nc.sync.dma_start(tv[:], bv_d[e * PT : (e + 1) * PT, :])
            bv_t.append(tv)

        qT_s = dramp.tile([D, S], BF16, tag="qts", name="qts")
        vT_t = [vTp.tile([PT, S], BF16, tag=f"vT{e}", name=f"vT{e}") for e in range(NTD)]

        # ---- Phase A/B: projections qT (to DRAM) and vT (resident) ----
        with (
            tc.tile_pool(name="xTp", bufs=1) as xTp,
            tc.tile_pool(name="qstage", bufs=2) as qstagep,
            tc.tile_pool(name="wblk", bufs=1) as wblkp,
            tc.tile_pool(name="psAB", bufs=2, space="PSUM") as psABp,
        ):
            xT_t = [xTp.tile([PT, S], BF16, tag=f"xT{k}", name=f"xT{k}") for k in range(NTD)]
            for k in range(NTD):
                nc.sync.dma_start(xT_t[k][:], xT_d[k * PT : (k + 1) * PT, :])
            Wv_t, Wq_t = [], []
            for k in range(NTD):
                wv = wblkp.tile([PT, D], BF16, tag=f"wv{k}", name=f"wv{k}")
                nc.sync.dma_start(wv[:], WvT_d[k * PT : (k + 1) * PT, :])
                Wv_t.append(wv)
                wq = wblkp.tile([PT, D], BF16, tag=f"wq{k}", name=f"wq{k}")
                nc.sync.dma_start(wq[:], WqT_d[k * PT : (k + 1) * PT, :])
                Wq_t.append(wq)

            for e in range(NTD):
                for which, W_t, b_t in (("v", Wv_t, bq_t), ("q", Wq_t, bq_t)):
                    W_t = Wv_t if which == "v" else Wq_t
                    b_t = bv_t if which == "v" else bq_t
                    ps = psABp.tile([PT, S], FP32, tag="ps", name="ps")
                    for k in range(NTD):
                        for sc in range(S // 512):
                            nc.tensor.matmul(
                                ps[:, sc * 512 : (sc + 1) * 512],
                                W_t[k][:, e * PT : (e + 1) * PT],
                                xT_t[k][:, sc * 512 : (sc + 1) * 512],
                                start=(k == 0),
                                stop=(k == NTD - 1),
                            )
                    if which == "v":
                        nc.scalar.activation(
                            vT_t[e][:], ps[:], ACT.Identity, bias=b_t[e][:], scale=1.0
                        )
                    else:
                        qs = qstagep.tile([PT, S], BF16, tag="qs", name="qs")
                        nc.scalar.activation(
                            qs[:], ps[:], ACT.Identity, bias=b_t[e][:], scale=1.0
                        )
                        nc.sync.dma_start(qT_s[e * PT : (e + 1) * PT, :], qs[:])

        # ---- Phase C: v (natural layout, bf16) = transpose(vT) ----
        with tc.tile_pool(name="vp", bufs=1) as vp:
            v_t = [vp.tile([PT, D], BF16, tag=f"v{k}", name=f"v{k}") for k in range(NTS)]
            with tc.tile_pool(name="tpps", bufs=4, space="PSUM") as tpps:
                for k in range(NTS):
                    for e in range(NTD):
                        tp = tpps.tile([PT, PT], BF16, tag="tp", name="tp")
                        nc.tensor.transpose(
                            tp[:], vT_t[e][:, k * PT : (k + 1) * PT], ident[:]
                        )
                        dst = v_t[k][:, e * PT : (e + 1) * PT]
                        if e % 2 == 0:
                            nc.vector.tensor_copy(dst, tp[:])
                        else:
                            nc.scalar.copy(dst, tp[:])

            # ---- Phase D: attention blocks ----
            with (
                tc.tile_pool(name="qTbp", bufs=1) as qTbp,
                tc.tile_pool(name="eTp", bufs=1) as eTp,
                tc.tile_pool(name="ctxp", bufs=1) as ctxp,
                tc.tile_pool(name="rlp", bufs=1) as rlp,
                tc.tile_pool(name="outp", bufs=2) as outp,
                tc.tile_pool(name="psS", bufs=2, space="PSUM") as psSp,
                tc.tile_pool(name="psC", bufs=2, space="PSUM") as psCp,
                tc.tile_pool(name="psL", bufs=2, space="PSUM") as psLp,
                tc.tile_pool(name="psO", bufs=2, space="PSUM") as psOp,
            ):
                for c in range(NBLK):
                    q0 = c * QB
                    kmax = 4 * (c + 1) if causal else NTS
                    qTb_t = []
                    for e in range(NTD):
                        qb = qTbp.tile([PT, QB], BF16, tag=f"qTb{e}", name=f"qTb{e}")
                        nc.sync.dma_start(
                            qb[:], qT_s[e * PT : (e + 1) * PT, q0 : q0 + QB]
                        )
                        qTb_t.append(qb)

                    eT_t = []
                    for ki in range(kmax):
                        ps = psSp.tile([PT, QB], FP32, tag="s", name="s")
                        for e in range(NTD):
                            nc.tensor.matmul(
                                ps[:],
                                vT_t[e][:, ki * PT : (ki + 1) * PT],
                                qTb_t[e][:],
                                start=(e == 0),
                                stop=(e == NTD - 1),
                            )
                        et = eTp.tile([PT, QB], BF16, tag=f"e{ki}", name=f"e{ki}")
                        if causal and ki >= 4 * c:
                            m = ki - 4 * c
                            if m > 0:
                                nc.vector.memset(et[:, 0 : m * PT], 0.0)
                            nc.scalar.activation(
                                et[:, m * PT : QB],
                                ps[:, m * PT : QB],
                                ACT.Exp,
                                scale=float(SCALE),
                            )
                            nc.vector.tensor_mul(
                                et[:, m * PT : (m + 1) * PT],
                                et[:, m * PT : (m + 1) * PT],
                                utri_t[:],
                            )
                        else:
                            nc.scalar.activation(
                                et[:], ps[:], ACT.Exp, scale=float(SCALE)
                            )
                        eT_t.append(et)

                    ctx_t = []
                    for d in range(NTD):
                        pc = psCp.tile([PT, QB], FP32, tag="c", name="c")
                        for ki in range(kmax):
                            nc.tensor.matmul(
                                pc[:],
                                v_t[ki][:, d * PT : (d + 1) * PT],
                                eT_t[ki][:],
                                start=(ki == 0),
                                stop=(ki == kmax - 1),
                            )
                        cx = ctxp.tile([PT, QB], BF16, tag=f"cx{d}", name=f"cx{d}")
                        nc.vector.tensor_copy(cx[:], pc[:])
                        ctx_t.append(cx)

                    rl_t = []
                    for qt in range(QB // PT):
                        pl = psLp.tile([PT, 1], FP32, tag="l", name="l")
                        for ki in range(kmax):
                            nc.tensor.matmul(
                                pl[:],
                                eT_t[ki][:, qt * PT : (qt + 1) * PT],
                                ones_t[:],
                                start=(ki == 0),
                                stop=(ki == kmax - 1),
                            )
                        r = rlp.tile([PT, 1], FP32, tag=f"rl{qt}", name=f"rl{qt}")
                        nc.vector.reciprocal(r[:], pl[:])
                        rl_t.append(r)

                    for qt in range(QB // PT):
                        os = outp.tile([PT, D], FP32, tag="os", name="os")
                        for ec in range(NEC):
                            po = psOp.tile([PT, 512], FP32, tag="o", name="o")
                            for d in range(NTD):
                                nc.tensor.matmul(
                                    po[:],
                                    ctx_t[d][:, qt * PT : (qt + 1) * PT],
                                    WoT_t[d][:, ec * 512 : (ec + 1) * 512],
                                    start=(d == 0),
                                    stop=(d == NTD - 1),
                                )
                            nc.vector.tensor_scalar_mul(
                                os[:, ec * 512 : (ec + 1) * 512], po[:], rl_t[qt][:]
                            )
                        nc.vector.tensor_add(os[:], os[:], bo_t[:])
                        nc.sync.dma_start(
                            out_d[q0 + qt * PT : q0 + (qt + 1) * PT, :], os[:]
                        )
    nc.finalize()
    return nc


_TRIL = None


def _detect_causal(mask: np.ndarray) -> bool:
    global _TRIL
    m0 = np.asarray(mask[0])
    if bool(m0[0, 1]):
        # upper-triangular entry set -> not causal; verify all-ones
        if not m0.all() or not np.asarray(mask).all():
            raise NotImplementedError("unsupported mask pattern")
        return False
    if _TRIL is None:
        _TRIL = np.tril(np.ones((S, S), dtype=bool))
    for b in range(mask.shape[0]):
        if not np.array_equal(np.asarray(mask[b]), _TRIL):
            raise NotImplementedError("unsupported mask pattern")
    return True


def kernel(x, mask, Wq, bq, Wk, bk, Wv, bv, Wo, bo):
    x = np.asarray(x, dtype=np.float32)
    causal = _detect_causal(np.asarray(mask))
    nc = build_nc(causal)

    import ml_dtypes
    WqT = np.ascontiguousarray(np.asarray(Wq, dtype=np.float32).T).astype(ml_dtypes.bfloat16)
    WvT = np.ascontiguousarray(np.asarray(Wv, dtype=np.float32).T).astype(ml_dtypes.bfloat16)
    WoT = np.ascontiguousarray(np.asarray(Wo, dtype=np.float32).T).astype(ml_dtypes.bfloat16)
    bqc = np.asarray(bq, dtype=np.float32).reshape(D, 1)
    bvc = np.asarray(bv, dtype=np.float32).reshape(D, 1)
    bob = np.tile(np.asarray(bo, dtype=np.float32).reshape(1, D), (PT, 1))
    ones_col = np.ones((PT, 1), dtype=np.float32)
    utri = np.triu(np.ones((PT, PT), dtype=np.float32))
    ones_col = ones_col.astype(ml_dtypes.bfloat16)
    utri = utri.astype(ml_dtypes.bfloat16)

    in_maps = []
    for b in range(B):
        in_maps.append(
            {
                "xT": np.ascontiguousarray(x[b].T).astype(ml_dtypes.bfloat16),
                "WqT": WqT,
                "WvT": WvT,
                "WoT": WoT,
                "bqc": bqc,
                "bvc": bvc,
                "bob": bob,
                "ones_col": ones_col,
                "utri": utri,
            }
        )
    res = run_bass_kernel_spmd(nc, in_maps, list(range(B)))
    out = np.stack([np.asarray(res.results[i]["out"]) for i in range(B)])
    return out.astype(np.float32)


if __name__ == "__main__":
    rng = np.random.default_rng(0)
    x = rng.standard_normal((B, S, D), dtype=np.float32)
    mask = np.broadcast_to(np.tril(np.ones((S, S), dtype=bool)), (B, S, S))
    mk = lambda *s: (rng.standard_normal(s, dtype=np.float32) * 0.02)
    out = kernel(
        x, mask, mk(D, D), mk(D), mk(D, D), mk(D), mk(D, D), mk(D), mk(D, D), mk(D)
    )
    print(out.shape, out.dtype)


# revision 16
# speedup vs baseline: 36.1117x; 36.1117x over previous
"""Trainium2 Bass kernel for nn_Attention_44564580663760.

Single-head "attention" (B=8, S=2048, D=1024, fp32) with the reference's
quirk reproduced: scores = q @ v^T (k projection unused), causal mask,
softmax, ctx @ v, output projection.

Sharding: data-parallel — one batch element per NeuronCore (8 cores).

Per-core dataflow (layouts chosen so the attention matrix never needs an
on-chip transpose; matmul contracts the partition dim):
    xT  [d, s]   (host-transposed input, bf16)
    qT  [e, s] = WqT.T @ xT (+bq)         -> DRAM scratch, reloaded per block
    vT  [e, s] = WvT.T @ xT (+bv)         -> resident
    v   [k, d] = PE-transpose(vT)         -> resident
    per q-block (512 cols):
      scoresT [k, q] = vT.T @ qT          (fp32 psum)
      eT = exp(scoresT/32) (bf16), causal-masked
      ctxT [d, q]: lhsT = v slices, rhs = eT
      l[q] = column sums of eT via N=1 matmuls with ones (lhsT = eT)
      out [q, e]: lhsT = ctxT slices, rhs = WoT; scaled by 1/l, + bo
"""

import sys

sys.path.insert(0, "/opt/trn_rl_repo")

import contextlib

import numpy as np

import concourse.bacc as bacc
import concourse.bass as bass
import concourse.mybir as mybir
import concourse.tile as tile
from concourse.bass_utils import run_bass_kernel_spmd
from concourse.masks import make_identity

FP32 = mybir.dt.float32
BF16 = mybir.dt.bfloat16

B, S, D = 8, 2048, 1024
PT = 128
NTS = S // PT  # 16 s-tiles
NTD = D // PT  # 8 d-tiles
QB = 512  # q-block width
NBLK = S // QB  # 4
KPB = QB // PT  # 4 k-tiles per block
SCALE = 1.0 / np.sqrt(np.float32(D))
ACT = mybir.ActivationFunctionType


def build_nc(causal: bool, reps: int = 0) -> bass.Bass:
    nc = bacc.Bacc("TRN2", target_bir_lowering=False, debug=False)
    dram = {
        "xT": nc.declare_dram_parameter("xT", [D, S], BF16, isOutput=False),
        "WqT": nc.declare_dram_parameter("WqT", [D, D], BF16, isOutput=False),
        "WvT": nc.declare_dram_parameter("WvT", [D, D], BF16, isOutput=False),
        "WoT": nc.declare_dram_parameter("WoT", [D, D], BF16, isOutput=False),
        "bqc": nc.declare_dram_parameter("bqc", [D, 1], FP32, isOutput=False),
        "bvc": nc.declare_dram_parameter("bvc", [D, 1], FP32, isOutput=False),
        "bob": nc.declare_dram_parameter("bob", [PT, D], FP32, isOutput=False),
        "ones_col": nc.declare_dram_parameter("ones_col", [PT, 1], BF16, isOutput=False),
        "utri": nc.declare_dram_parameter("utri", [PT, PT], BF16, isOutput=False),
        "out": nc.declare_dram_parameter("out", [S, D], FP32, isOutput=True),
    }

    with (
        tile.TileContext(nc) as tc,
        tc.tile_pool(name="const", bufs=1) as constp,
        tc.tile_pool(name="dramp", bufs=1, space="DRAM") as dramp,
        tc.tile_pool(name="vTp", bufs=1) as vTp,
    ):
        qT_s = dramp.tile([D, S], BF16, tag="qts", name="qts")
        vT_t = [
            vTp.tile([PT, S], BF16, tag=f"vT{e}", name=f"vT{e}") for e in range(NTD)
        ]
        loop_ctx = tc.For_i(0, reps, 1) if reps else contextlib.nullcontext()
        with loop_ctx:
            _body(nc, tc, causal, constp, dram, qT_s, vT_t)
    nc.finalize()
    return nc


def _body(nc, tc, causal, constp, dram, qT_s, vT_t):
    with (
        tc.tile_pool(name="xTp", bufs=1) as xTp,
        tc.tile_pool(name="qstage", bufs=2) as qstagep,
        tc.tile_pool(name="wblk", bufs=1) as wblkp,
        tc.tile_pool(name="psAB", bufs=2, space="PSUM") as psABp,
    ):
        # projection-critical loads first, interleaved so (xT[k], Wv[k], Wq[k])
        # arrive together in k order
        xT_t, Wv_t, Wq_t = [], [], []
        for k in range(NTD):
            xt = xTp.tile([PT, S], BF16, tag=f"xT{k}", name=f"xT{k}")
            nc.sync.dma_start(xt[:], dram["xT"][k * PT : (k + 1) * PT, :])
            xT_t.append(xt)
            wv = wblkp.tile([PT, D], BF16, tag=f"wv{k}", name=f"wv{k}")
            nc.sync.dma_start(wv[:], dram["WvT"][k * PT : (k + 1) * PT, :])
            Wv_t.append(wv)
            wq = wblkp.tile([PT, D], BF16, tag=f"wq{k}", name=f"wq{k}")
            nc.sync.dma_start(wq[:], dram["WqT"][k * PT : (k + 1) * PT, :])
            Wq_t.append(wq)
        bq_t, bv_t = [], []
        for e in range(NTD):
            tq = constp.tile([PT, 1], FP32, tag=f"bq{e}", name=f"bq{e}")
            nc.sync.dma_start(tq[:], dram["bqc"][e * PT : (e + 1) * PT, :])
            bq_t.append(tq)
            tv = constp.tile([PT, 1], FP32, tag=f"bv{e}", name=f"bv{e}")
            nc.sync.dma_start(tv[:], dram["bvc"][e * PT : (e + 1) * PT, :])
            bv_t.append(tv)

        # ---- Phase A/B: projections vT (resident) and qT (to DRAM) ----
        for e in range(NTD):
            for which in ("v", "q"):
                W_t = Wv_t if which == "v" else Wq_t
                b_t = bv_t if which == "v" else bq_t
                ps = psABp.tile([PT, S], FP32, tag="ps", name="ps")
                for k in range(NTD):
                    for sc in range(S // 512):
                        nc.tensor.matmul(
                            ps[:, sc * 512 : (sc + 1) * 512],
                            W_t[k][:, e * PT : (e + 1) * PT],
                            xT_t[k][:, sc * 512 : (sc + 1) * 512],
                            start=(k == 0),
                            stop=(k == NTD - 1),
                        )
                if which == "v":
                    nc.scalar.activation(
                        vT_t[e][:], ps[:], ACT.Identity, bias=b_t[e][:], scale=1.0
                    )
                else:
                    qs = qstagep.tile([PT, S], BF16, tag="qs", name="qs")
                    nc.scalar.activation(
                        qs[:], ps[:], ACT.Identity, bias=b_t[e][:], scale=1.0
                    )
                    nc.sync.dma_start(qT_s[e * PT : (e + 1) * PT, :], qs[:])

    # constants for phases C/D (emitted late so they don't delay xT/W loads)
    ident = constp.tile([PT, PT], BF16, tag="ident", name="ident")
    make_identity(nc, ident[:])
    WoT_t = []
    for d in range(NTD):
        w = constp.tile([PT, D], BF16, tag=f"wot{d}", name=f"wot{d}")
        nc.sync.dma_start(w[:], dram["WoT"][d * PT : (d + 1) * PT, :])
        WoT_t.append(w)
    bo_t = constp.tile([PT, D], FP32, tag="bo", name="bo")
    nc.sync.dma_start(bo_t[:], dram["bob"][:, :])
    ones_t = constp.tile([PT, 1], BF16, tag="ones", name="ones")
    nc.sync.dma_start(ones_t[:], dram["ones_col"][:, :])
    utri_t = constp.tile([PT, PT], BF16, tag="utri", name="utri")
    nc.sync.dma_start(utri_t[:], dram["utri"][:, :])

    # ---- Phase C: v (natural layout) = PE-transpose(vT) ----
    with tc.tile_pool(name="vp", bufs=1) as vp:
        v_t = [vp.tile([PT, D], BF16, tag=f"v{k}", name=f"v{k}") for k in range(NTS)]
        with tc.tile_pool(name="tpps", bufs=4, space="PSUM") as tpps:
            for k in range(NTS):
                for e in range(NTD):
                    tp = tpps.tile([PT, PT], BF16, tag="tp", name="tp")
                    nc.tensor.transpose(
                        tp[:], vT_t[e][:, k * PT : (k + 1) * PT], ident[:]
                    )
                    dst = v_t[k][:, e * PT : (e + 1) * PT]
                    if e % 2 == 0:
                        nc.vector.tensor_copy(dst, tp[:])
                    else:
                        nc.scalar.copy(dst, tp[:])

        # ---- Phase D: attention q-blocks ----
        with (
            tc.tile_pool(name="qTbp", bufs=2) as qTbp,
            tc.tile_pool(name="eTp", bufs=1) as eTp,
            tc.tile_pool(name="ctxp", bufs=1) as ctxp,
            tc.tile_pool(name="rlp", bufs=1) as rlp,
            tc.tile_pool(name="outp", bufs=2) as outp,
            tc.tile_pool(name="psS", bufs=2, space="PSUM") as psSp,
            tc.tile_pool(name="psC", bufs=2, space="PSUM") as psCp,
            tc.tile_pool(name="psL", bufs=2, space="PSUM") as psLp,
            tc.tile_pool(name="psO", bufs=2, space="PSUM") as psOp,
        ):

            def load_qTb(c):
                q0 = c * QB
                tiles = []
                for e in range(NTD):
                    qb = qTbp.tile([PT, QB], BF16, tag=f"qTb{e}", name=f"qTb{e}")
                    nc.sync.dma_start(
                        qb[:], qT_s[e * PT : (e + 1) * PT, q0 : q0 + QB]
                    )
                    tiles.append(qb)
                return tiles

            qTb_cur = load_qTb(0)
            for c in range(NBLK):
                q0 = c * QB
                kmax = KPB * (c + 1) if causal else NTS

                # scoresT + exp -> eT tiles (ragged in the diagonal region)
                eT_t = []
                for ki in range(kmax):
                    m = ki - KPB * c  # >=0 in diagonal region
                    lo = m * PT if (causal and m > 0) else 0
                    ps = psSp.tile([PT, QB], FP32, tag="s", name="s")
                    for e in range(NTD):
                        nc.tensor.matmul(
                            ps[:, lo:QB],
                            vT_t[e][:, ki * PT : (ki + 1) * PT],
                            qTb_cur[e][:, lo:QB],
                            start=(e == 0),
                            stop=(e == NTD - 1),
                        )
                    et = eTp.tile([PT, QB], BF16, tag=f"e{ki}", name=f"e{ki}")
                    if causal and m >= 0:
                        if m > 0:
                            nc.gpsimd.memset(et[:, 0:lo], 0.0)
                        nc.scalar.activation(
                            et[:, lo:QB], ps[:, lo:QB], ACT.Exp, scale=float(SCALE)
                        )
                        nc.vector.tensor_mul(
                            et[:, m * PT : (m + 1) * PT],
                            et[:, m * PT : (m + 1) * PT],
                            utri_t[:],
                        )
                    else:
                        nc.scalar.activation(et[:], ps[:], ACT.Exp, scale=float(SCALE))
                    eT_t.append(et)

                # prefetch next block's qT while PE chews on ctx/out
                if c + 1 < NBLK:
                    qTb_next = load_qTb(c + 1)

                # ctxT[d, q-block], accumulated over k tiles (ragged on diag)
                ctx_t = []
                for d in range(NTD):
                    pc = psCp.tile([PT, QB], FP32, tag="c", name="c")
                    for ki in range(kmax):
                        m = ki - KPB * c
                        lo = m * PT if (causal and m > 0) else 0
                        nc.tensor.matmul(
                            pc[:, lo:QB],
                            v_t[ki][:, d * PT : (d + 1) * PT],
                            eT_t[ki][:, lo:QB],
                            start=(ki == 0),
                            stop=(ki == kmax - 1),
                        )
                    cx = ctxp.tile([PT, QB], BF16, tag=f"cx{d}", name=f"cx{d}")
                    nc.vector.tensor_copy(cx[:], pc[:])
                    ctx_t.append(cx)

                # softmax denominators per q sub-tile: l = eT.T @ ones
                rl_t = []
                for qt in range(KPB):
                    pl = psLp.tile([PT, 1], FP32, tag="l", name="l")
                    for ki in range(kmax):
                        nc.tensor.matmul(
                            pl[:],
                            eT_t[ki][:, qt * PT : (qt + 1) * PT],
                            ones_t[:],
                            start=(ki == 0),
                            stop=(ki == kmax - 1),
                        )
                    r = rlp.tile([PT, 1], FP32, tag=f"rl{qt}", name=f"rl{qt}")
                    nc.vector.reciprocal(r[:], pl[:])
                    rl_t.append(r)

                # out projection + normalize + bias + store
                for qt in range(KPB):
                    os_ = outp.tile([PT, D], FP32, tag="os", name="os")
                    for ec in range(D // 512):
                        po = psOp.tile([PT, 512], FP32, tag="o", name="o")
                        for d in range(NTD):
                            nc.tensor.matmul(
                                po[:],
                                ctx_t[d][:, qt * PT : (qt + 1) * PT],
                                WoT_t[d][:, ec * 512 : (ec + 1) * 512],
                                start=(d == 0),
                                stop=(d == NTD - 1),
                            )
                        nc.vector.tensor_scalar_mul(
                            os_[:, ec * 512 : (ec + 1) * 512], po[:], rl_t[qt][:]
                        )
                    nc.vector.tensor_add(os_[:], os_[:], bo_t[:])
                    nc.sync.dma_start(
                        dram["out"][q0 + qt * PT : q0 + (qt + 1) * PT, :], os_[:]
                    )
                if c + 1 < NBLK:
                    qTb_cur = qTb_next


_TRIL = None


def _detect_causal(mask: np.ndarray) -> bool:
    global _TRIL
    m0 = np.asarray(mask[0])
    if bool(m0[0, 1]):
        if not m0.all() or not np.asarray(mask).all():
            raise NotImplementedError("unsupported mask pattern")
        return False
    if _TRIL is None:
        _TRIL = np.tril(np.ones((S, S), dtype=bool))
    for b in range(mask.shape[0]):
        if not np.array_equal(np.asarray(mask[b]), _TRIL):
            raise NotImplementedError("unsupported mask pattern")
    return True


def kernel(x, mask, Wq, bq, Wk, bk, Wv, bv, Wo, bo):
    import ml_dtypes

    x = np.asarray(x, dtype=np.float32)
    causal = _detect_causal(np.asarray(mask))
    nc = build_nc(causal)

    WqT = np.ascontiguousarray(np.asarray(Wq, dtype=np.float32).T).astype(
        ml_dtypes.bfloat16
    )
    WvT = np.ascontiguousarray(np.asarray(Wv, dtype=np.float32).T).astype(
        ml_dtypes.bfloat16
    )
    WoT = np.ascontiguousarray(np.asarray(Wo, dtype=np.float32).T).astype(
        ml_dtypes.bfloat16
    )
    base = {
        "WqT": WqT,
        "WvT": WvT,
        "WoT": WoT,
        "bqc": np.asarray(bq, dtype=np.float32).reshape(D, 1),
        "bvc": np.asarray(bv, dtype=np.float32).reshape(D, 1),
        "bob": np.tile(np.asarray(bo, dtype=np.float32).reshape(1, D), (PT, 1)),
        "ones_col": np.ones((PT, 1), dtype=ml_dtypes.bfloat16),
        "utri": np.triu(np.ones((PT, PT), dtype=np.float32)).astype(ml_dtypes.bfloat16),
    }
    in_maps = [
        {"xT": np.ascontiguousarray(x[b].T).astype(ml_dtypes.bfloat16), **base}
        for b in range(B)
    ]
    res = run_bass_kernel_spmd(nc, in_maps, list(range(B)))
    out = np.stack([np.asarray(res.results[i]["out"]) for i in range(B)])
    return out.astype(np.float32)


if __name__ == "__main__":
    rng = np.random.default_rng(0)
    x = rng.standard_normal((B, S, D), dtype=np.float32)
    mask = np.broadcast_to(np.tril(np.ones((S, S), dtype=bool)), (B, S, S))
    mk = lambda *s: (rng.standard_normal(s, dtype=np.float32) * 0.02)
    out = kernel(
        x, mask, mk(D, D), mk(D), mk(D, D), mk(D), mk(D, D), mk(D), mk(D, D), mk(D)
    )
    print(out.shape, out.dtype)


# revision 17
# speedup vs baseline: 85.7881x; 2.3756x over previous
"""Trainium2 Bass kernel for nn_Attention_44564580663760.

Single-head "attention" (B=8, S=2048, D=1024, fp32) with the reference's
quirk reproduced: scores = q @ v^T (k projection unused), causal mask,
softmax, ctx @ v, output projection.

Sharding: data-parallel — one batch element per NeuronCore (8 cores).

Per-core dataflow (layouts chosen so the attention matrix never needs an
on-chip transpose; matmul contracts the partition dim):
    xT  [d, s]   (host-transposed input, bf16)
    qT  [e, s] = WqT.T @ xT (+bq)         -> DRAM scratch, reloaded per block
    vT  [e, s] = WvT.T @ xT (+bv)         -> resident
    v   [k, d] = PE-transpose(vT)         -> resident
    per q-block (512 cols):
      scoresT [k, q] = vT.T @ qT          (fp32 psum)
      eT = exp(scoresT/32) (bf16), causal-masked
      ctxT [d, q]: lhsT = v slices, rhs = eT
      l[q] = column sums of eT via N=1 matmuls with ones (lhsT = eT)
      out [q, e]: lhsT = ctxT slices, rhs = WoT; scaled by 1/l, + bo
"""

import sys

sys.path.insert(0, "/opt/trn_rl_repo")

import contextlib

import numpy as np

import concourse.bacc as bacc
import concourse.bass as bass
import concourse.mybir as mybir
import concourse.tile as tile
from concourse.bass_utils import run_bass_kernel_spmd
from concourse.masks import make_identity

FP32 = mybir.dt.float32
BF16 = mybir.dt.bfloat16

B, S, D = 8, 2048, 1024
PT = 128
NTS = S // PT  # 16 s-tiles
NTD = D // PT  # 8 d-tiles
QB = 512  # q-block width
NBLK = S // QB  # 4
KPB = QB // PT  # 4 k-tiles per block
SCALE = 1.0 / np.sqrt(np.float32(D))
ACT = mybir.ActivationFunctionType


def build_nc(causal: bool, reps: int = 0) -> bass.Bass:
    nc = bacc.Bacc("TRN2", target_bir_lowering=False, debug=False)
    dram = {
        "xT": nc.declare_dram_parameter("xT", [D, S], BF16, isOutput=False),
        "WqT": nc.declare_dram_parameter("WqT", [D, D], BF16, isOutput=False),
        "WvT": nc.declare_dram_parameter("WvT", [D, D], BF16, isOutput=False),
        "WoT": nc.declare_dram_parameter("WoT", [D, D], BF16, isOutput=False),
        "bqc": nc.declare_dram_parameter("bqc", [D, 1], FP32, isOutput=False),
        "bvc": nc.declare_dram_parameter("bvc", [D, 1], FP32, isOutput=False),
        "bob": nc.declare_dram_parameter("bob", [PT, D], FP32, isOutput=False),
        "ones_col": nc.declare_dram_parameter("ones_col", [PT, 1], BF16, isOutput=False),
        "utri": nc.declare_dram_parameter("utri", [PT, PT], BF16, isOutput=False),
        "out": nc.declare_dram_parameter("out", [S, D], FP32, isOutput=True),
    }

    with (
        tile.TileContext(nc) as tc,
        tc.tile_pool(name="const", bufs=1) as constp,
        tc.tile_pool(name="dramp", bufs=1, space="DRAM") as dramp,
        tc.tile_pool(name="vTp", bufs=1) as vTp,
    ):
        qT_s = dramp.tile([D, S], BF16, tag="qts", name="qts")
        vT_t = [
            vTp.tile([PT, S], BF16, tag=f"vT{e}", name=f"vT{e}") for e in range(NTD)
        ]
        loop_ctx = tc.For_i(0, reps, 1) if reps else contextlib.nullcontext()
        with loop_ctx:
            _body(nc, tc, causal, constp, dram, qT_s, vT_t)
    _dedup_ldweights(nc)
    nc.finalize()
    return nc


def _dedup_ldweights(nc):
    """Drop InstLdweights whose stationary operand matches the previous PE
    weight load (no intervening PE weight change) — the paired matmuls then
    reuse the already-loaded weights. Deps of a dropped LDW move to the next
    kept instruction so semaphore generation still orders correctly."""
    removed = {}
    n_drop = 0
    for bb in nc.main_func.blocks:
        insts = bb.instructions
        keep = []
        last_sig = None
        pending = []
        for ins in insts:
            drop = False
            if isinstance(ins, mybir.InstLdweights):
                sig = (
                    str(ins.ins[0]),
                    bool(ins.is_transpose),
                    str(ins.perf_mode),
                    str(ins.tile_position),
                )
                if sig == last_sig:
                    drop = True
                else:
                    last_sig = sig
            elif (
                getattr(ins, "engine", None) == mybir.EngineType.PE
                and isinstance(ins, mybir.InstMatmult)
                and ins.is_transpose
            ):
                # transpose-mode matmuls change the loaded weights
                last_sig = None
            if drop:
                pending.append(ins)
                n_drop += 1
                continue
            for p in pending:
                ins.merge_dependencies_from(p)
                removed[p.name] = ins.name
            pending = []
            keep.append(ins)
        assert not pending
        if len(keep) != len(insts):
            insts[:] = keep
    if removed:
        for bb in nc.main_func.blocks:
            for ins in bb.instructions:
                ins.remap_dependency_names(removed)
        if hasattr(nc, "inst_map"):
            for name in removed:
                nc.inst_map.pop(name, None)


def _body(nc, tc, causal, constp, dram, qT_s, vT_t):
    with (
        tc.tile_pool(name="xTp", bufs=1) as xTp,
        tc.tile_pool(name="qstage", bufs=2) as qstagep,
        tc.tile_pool(name="wblk", bufs=1) as wblkp,
        tc.tile_pool(name="psAB", bufs=2, space="PSUM") as psABp,
    ):
        # projection-critical loads first, interleaved so (xT[k], Wv[k], Wq[k])
        # arrive together in k order
        xT_t, Wv_t, Wq_t = [], [], []
        for k in range(NTD):
            xt = xTp.tile([PT, S], BF16, tag=f"xT{k}", name=f"xT{k}")
            nc.sync.dma_start(xt[:], dram["xT"][k * PT : (k + 1) * PT, :])
            xT_t.append(xt)
            wv = wblkp.tile([PT, D], BF16, tag=f"wv{k}", name=f"wv{k}")
            nc.sync.dma_start(wv[:], dram["WvT"][k * PT : (k + 1) * PT, :])
            Wv_t.append(wv)
            wq = wblkp.tile([PT, D], BF16, tag=f"wq{k}", name=f"wq{k}")
            nc.sync.dma_start(wq[:], dram["WqT"][k * PT : (k + 1) * PT, :])
            Wq_t.append(wq)
        bq_t, bv_t = [], []
        for e in range(NTD):
            tq = constp.tile([PT, 1], FP32, tag=f"bq{e}", name=f"bq{e}")
            nc.sync.dma_start(tq[:], dram["bqc"][e * PT : (e + 1) * PT, :])
            bq_t.append(tq)
            tv = constp.tile([PT, 1], FP32, tag=f"bv{e}", name=f"bv{e}")
            nc.sync.dma_start(tv[:], dram["bvc"][e * PT : (e + 1) * PT, :])
            bv_t.append(tv)

        # ---- Phase A/B: projections vT (resident) and qT (to DRAM) ----
        for e in range(NTD):
            for which in ("v", "q"):
                W_t = Wv_t if which == "v" else Wq_t
                b_t = bv_t if which == "v" else bq_t
                ps = psABp.tile([PT, S], FP32, tag="ps", name="ps")
                for k in range(NTD):
                    for sc in range(S // 512):
                        nc.tensor.matmul(
                            ps[:, sc * 512 : (sc + 1) * 512],
                            W_t[k][:, e * PT : (e + 1) * PT],
                            xT_t[k][:, sc * 512 : (sc + 1) * 512],
                            start=(k == 0),
                            stop=(k == NTD - 1),
                        )
                if which == "v":
                    nc.scalar.activation(
                        vT_t[e][:], ps[:], ACT.Identity, bias=b_t[e][:], scale=1.0
                    )
                else:
                    qs = qstagep.tile([PT, S], BF16, tag="qs", name="qs")
                    nc.scalar.activation(
                        qs[:], ps[:], ACT.Identity, bias=b_t[e][:], scale=1.0
                    )
                    nc.sync.dma_start(qT_s[e * PT : (e + 1) * PT, :], qs[:])

    # constants for phases C/D (emitted late so they don't delay xT/W loads)
    ident = constp.tile([PT, PT], BF16, tag="ident", name="ident")
    make_identity(nc, ident[:])
    WoT_t = []
    for d in range(NTD):
        w = constp.tile([PT, D], BF16, tag=f"wot{d}", name=f"wot{d}")
        nc.sync.dma_start(w[:], dram["WoT"][d * PT : (d + 1) * PT, :])
        WoT_t.append(w)
    bo_t = constp.tile([PT, D], FP32, tag="bo", name="bo")
    nc.sync.dma_start(bo_t[:], dram["bob"][:, :])
    ones_t = constp.tile([PT, 1], BF16, tag="ones", name="ones")
    nc.sync.dma_start(ones_t[:], dram["ones_col"][:, :])
    utri_t = constp.tile([PT, PT], BF16, tag="utri", name="utri")
    nc.sync.dma_start(utri_t[:], dram["utri"][:, :])

    # ---- Phase C: v (natural layout) = PE-transpose(vT) ----
    with tc.tile_pool(name="vp", bufs=1) as vp:
        v_t = [vp.tile([PT, D], BF16, tag=f"v{k}", name=f"v{k}") for k in range(NTS)]
        with tc.tile_pool(name="tpps", bufs=4, space="PSUM") as tpps:
            for k in range(NTS):
                for e in range(NTD):
                    tp = tpps.tile([PT, PT], BF16, tag="tp", name="tp")
                    nc.tensor.transpose(
                        tp[:], vT_t[e][:, k * PT : (k + 1) * PT], ident[:]
                    )
                    dst = v_t[k][:, e * PT : (e + 1) * PT]
                    if e % 2 == 0:
                        nc.vector.tensor_copy(dst, tp[:])
                    else:
                        nc.scalar.copy(dst, tp[:])

        # ---- Phase D: attention q-blocks ----
        with (
            tc.tile_pool(name="qTbp", bufs=2) as qTbp,
            tc.tile_pool(name="eTp", bufs=1) as eTp,
            tc.tile_pool(name="ctxp", bufs=1) as ctxp,
            tc.tile_pool(name="rlp", bufs=1) as rlp,
            tc.tile_pool(name="outp", bufs=2) as outp,
            tc.tile_pool(name="psS", bufs=2, space="PSUM") as psSp,
            tc.tile_pool(name="psC", bufs=2, space="PSUM") as psCp,
            tc.tile_pool(name="psL", bufs=2, space="PSUM") as psLp,
            tc.tile_pool(name="psO", bufs=2, space="PSUM") as psOp,
        ):

            def load_qTb(c):
                q0 = c * QB
                tiles = []
                for e in range(NTD):
                    qb = qTbp.tile([PT, QB], BF16, tag=f"qTb{e}", name=f"qTb{e}")
                    nc.sync.dma_start(
                        qb[:], qT_s[e * PT : (e + 1) * PT, q0 : q0 + QB]
                    )
                    tiles.append(qb)
                return tiles

            qTb_cur = load_qTb(0)
            for c in range(NBLK):
                q0 = c * QB
                kmax = KPB * (c + 1) if causal else NTS

                # scoresT + exp -> eT tiles (ragged in the diagonal region)
                eT_t = []
                for ki in range(kmax):
                    m = ki - KPB * c  # >=0 in diagonal region
                    lo = m * PT if (causal and m > 0) else 0
                    ps = psSp.tile([PT, QB], FP32, tag="s", name="s")
                    for e in range(NTD):
                        nc.tensor.matmul(
                            ps[:, lo:QB],
                            vT_t[e][:, ki * PT : (ki + 1) * PT],
                            qTb_cur[e][:, lo:QB],
                            start=(e == 0),
                            stop=(e == NTD - 1),
                        )
                    et = eTp.tile([PT, QB], BF16, tag=f"e{ki}", name=f"e{ki}")
                    if causal and m >= 0:
                        if m > 0:
                            nc.gpsimd.memset(et[:, 0:lo], 0.0)
                        nc.scalar.activation(
                            et[:, lo:QB], ps[:, lo:QB], ACT.Exp, scale=float(SCALE)
                        )
                        nc.vector.tensor_mul(
                            et[:, m * PT : (m + 1) * PT],
                            et[:, m * PT : (m + 1) * PT],
                            utri_t[:],
                        )
                    else:
                        nc.scalar.activation(et[:], ps[:], ACT.Exp, scale=float(SCALE))
                    eT_t.append(et)

                # prefetch next block's qT while PE chews on ctx/out
                if c + 1 < NBLK:
                    qTb_next = load_qTb(c + 1)

                # ctxT[d, q-block], accumulated over k tiles (ragged on diag)
                ctx_t = []
                for d in range(NTD):
                    pc = psCp.tile([PT, QB], FP32, tag="c", name="c")
                    for ki in range(kmax):
                        m = ki - KPB * c
                        lo = m * PT if (causal and m > 0) else 0
                        nc.tensor.matmul(
                            pc[:, lo:QB],
                            v_t[ki][:, d * PT : (d + 1) * PT],
                            eT_t[ki][:, lo:QB],
                            start=(ki == 0),
                            stop=(ki == kmax - 1),
                        )
                    cx = ctxp.tile([PT, QB], BF16, tag=f"cx{d}", name=f"cx{d}")
                    nc.vector.tensor_copy(cx[:], pc[:])
                    ctx_t.append(cx)

                # softmax denominators per q sub-tile: l = eT.T @ ones
                rl_t = []
                for qt in range(KPB):
                    pl = psLp.tile([PT, 1], FP32, tag="l", name="l")
                    for ki in range(kmax):
                        nc.tensor.matmul(
                            pl[:],
                            eT_t[ki][:, qt * PT : (qt + 1) * PT],
                            ones_t[:],
                            start=(ki == 0),
                            stop=(ki == kmax - 1),
                        )
                    r = rlp.tile([PT, 1], FP32, tag=f"rl{qt}", name=f"rl{qt}")
                    nc.vector.reciprocal(r[:], pl[:])
                    rl_t.append(r)

                # out projection + normalize + bias + store
                for qt in range(KPB):
                    os_ = outp.tile([PT, D], FP32, tag="os", name="os")
                    pos = [
                        psOp.tile([PT, 512], FP32, tag="o", name="o")
                        for _ in range(D // 512)
                    ]
                    for d in range(NTD):
                        for ec in range(D // 512):
                            nc.tensor.matmul(
                                pos[ec][:],
                                ctx_t[d][:, qt * PT : (qt + 1) * PT],
                                WoT_t[d][:, ec * 512 : (ec + 1) * 512],
                                start=(d == 0),
                                stop=(d == NTD - 1),
                            )
                    for ec in range(D // 512):
                        nc.vector.tensor_scalar_mul(
                            os_[:, ec * 512 : (ec + 1) * 512], pos[ec][:], rl_t[qt][:]
                        )
                    nc.vector.tensor_add(os_[:], os_[:], bo_t[:])
                    nc.sync.dma_start(
                        dram["out"][q0 + qt * PT : q0 + (qt + 1) * PT, :], os_[:]
                    )
                if c + 1 < NBLK:
                    qTb_cur = qTb_next


_TRIL = None


def _detect_causal(mask: np.ndarray) -> bool:
    global _TRIL
    m0 = np.asarray(mask[0])
    if bool(m0[0, 1]):
        if not m0.all() or not np.asarray(mask).all():
            raise NotImplementedError("unsupported mask pattern")
        return False
    if _TRIL is None:
        _TRIL = np.tril(np.ones((S, S), dtype=bool))
    for b in range(mask.shape[0]):
        if not np.array_equal(np.asarray(mask[b]), _TRIL):
            raise NotImplementedError("unsupported mask pattern")
    return True


def kernel(x, mask, Wq, bq, Wk, bk, Wv, bv, Wo, bo):
    import ml_dtypes

    x = np.asarray(x, dtype=np.float32)
    causal = _detect_causal(np.asarray(mask))
    nc = build_nc(causal)

    WqT = np.ascontiguousarray(np.asarray(Wq, dtype=np.float32).T).astype(
        ml_dtypes.bfloat16
    )
    WvT = np.ascontiguousarray(np.asarray(Wv, dtype=np.float32).T).astype(
        ml_dtypes.bfloat16
    )
    WoT = np.ascontiguousarray(np.asarray(Wo, dtype=np.float32).T).astype(
        ml_dtypes.bfloat16
    )
    base = {
        "WqT": WqT,
        "WvT": WvT,
        "WoT": WoT,
        "bqc": np.asarray(bq, dtype=np.float32).reshape(D, 1),
        "bvc": np.asarray(bv, dtype=np.float32).reshape(D, 1),
        "bob": np.tile(np.asarray(bo, dtype=np.float32).reshape(1, D), (PT, 1)),
        "ones_col": np.ones((PT, 1), dtype=ml_dtypes.bfloat16),
        "utri": np.triu(np.ones((PT, PT), dtype=np.float32)).astype(ml_dtypes.bfloat16),
    }
    in_maps = [
        {"xT": np.ascontiguousarray(x[b].T).astype(ml_dtypes.bfloat16), **base}
        for b in range(B)
    ]
    res = run_bass_kernel_spmd(nc, in_maps, list(range(B)))
    out = np.stack([np.asarray(res.results[i]["out"]) for i in range(B)])
    return out.astype(np.float32)


if __name__ == "__main__":
    rng = np.random.default_rng(0)
    x = rng.standard_normal((B, S, D), dtype=np.float32)
    mask = np.broadcast_to(np.tril(np.ones((S, S), dtype=bool)), (B, S, S))
    mk = lambda *s: (rng.standard_normal(s, dtype=np.float32) * 0.02)
    out = kernel(
        x, mask, mk(D, D), mk(D), mk(D, D), mk(D), mk(D, D), mk(D), mk(D, D), mk(D)
    )
    print(out.shape, out.dtype)


# revision 18
# speedup vs baseline: 90.9382x; 1.0600x over previous
"""Trainium2 Bass kernel for nn_Attention_44564580663760.

Single-head "attention" (B=8, S=2048, D=1024, fp32) with the reference's
quirk reproduced: scores = q @ v^T (k projection unused), causal mask,
softmax, ctx @ v, output projection.

Sharding: data-parallel — one batch element per NeuronCore (8 cores).

Per-core dataflow (layouts chosen so the attention matrix never needs an
on-chip transpose; matmul contracts the partition dim):
    xT  [d, s]   (host-transposed input, bf16)
    qT  [e, s] = WqT.T @ xT (+bq)         -> DRAM scratch, reloaded per block
    vT  [e, s] = WvT.T @ xT (+bv)         -> resident
    v   [k, d] = PE-transpose(vT)         -> resident
    per q-block (512 cols):
      scoresT [k, q] = vT.T @ qT          (fp32 psum)
      eT = exp(scoresT/32) (bf16), causal-masked
      ctxT [d, q]: lhsT = v slices, rhs = eT
      l[q] = column sums of eT via N=1 matmuls with ones (lhsT = eT)
      out [q, e]: lhsT = ctxT slices, rhs = WoT; scaled by 1/l, + bo
"""

import sys

sys.path.insert(0, "/opt/trn_rl_repo")

import contextlib

import numpy as np

import concourse.bacc as bacc
import concourse.bass as bass
import concourse.mybir as mybir
import concourse.tile as tile
from concourse.bass_utils import run_bass_kernel_spmd
from concourse.masks import make_identity

FP32 = mybir.dt.float32
BF16 = mybir.dt.bfloat16

B, S, D = 8, 2048, 1024
PT = 128
NTS = S // PT  # 16 s-tiles
NTD = D // PT  # 8 d-tiles
QB = 512  # q-block width
NBLK = S // QB  # 4
KPB = QB // PT  # 4 k-tiles per block
SCALE = 1.0 / np.sqrt(np.float32(D))
ACT = mybir.ActivationFunctionType


def build_nc(causal: bool, reps: int = 0) -> bass.Bass:
    nc = bacc.Bacc("TRN2", target_bir_lowering=False, debug=False)
    dram = {
        "xT": nc.declare_dram_parameter("xT", [D, S], BF16, isOutput=False),
        "WqT": nc.declare_dram_parameter("WqT", [D, D], BF16, isOutput=False),
        "WvT": nc.declare_dram_parameter("WvT", [D, D], BF16, isOutput=False),
        "WoT": nc.declare_dram_parameter("WoT", [D, D], BF16, isOutput=False),
        "bqc": nc.declare_dram_parameter("bqc", [D, 1], FP32, isOutput=False),
        "bvc": nc.declare_dram_parameter("bvc", [D, 1], FP32, isOutput=False),
        "bob": nc.declare_dram_parameter("bob", [PT, D], FP32, isOutput=False),
        "ones_col": nc.declare_dram_parameter("ones_col", [PT, 1], BF16, isOutput=False),
        "utri": nc.declare_dram_parameter("utri", [PT, PT], BF16, isOutput=False),
        "out": nc.declare_dram_parameter("out", [S, D], FP32, isOutput=True),
    }

    with (
        tile.TileContext(nc) as tc,
        tc.tile_pool(name="const", bufs=1) as constp,
        tc.tile_pool(name="dramp", bufs=1, space="DRAM") as dramp,
        tc.tile_pool(name="vTp", bufs=1) as vTp,
    ):
        qT_s = dramp.tile([D, S], BF16, tag="qts", name="qts")
        vT_s = dramp.tile([D, S], BF16, tag="vts", name="vts")
        vT_t = [
            vTp.tile([PT, S], BF16, tag=f"vT{e}", name=f"vT{e}") for e in range(NTD)
        ]
        loop_ctx = tc.For_i(0, reps, 1) if reps else contextlib.nullcontext()
        with loop_ctx:
            _body(nc, tc, causal, constp, dram, qT_s, vT_s, vT_t)
    _dedup_ldweights(nc)
    nc.finalize()
    return nc


def _dedup_ldweights(nc):
    """Drop InstLdweights whose stationary operand matches the previous PE
    weight load (no intervening PE weight change) — the paired matmuls then
    reuse the already-loaded weights. Deps of a dropped LDW move to the next
    kept instruction so semaphore generation still orders correctly."""
    removed = {}
    n_drop = 0
    for bb in nc.main_func.blocks:
        insts = bb.instructions
        keep = []
        last_sig = None
        pending = []
        for ins in insts:
            drop = False
            if isinstance(ins, mybir.InstLdweights):
                sig = (
                    str(ins.ins[0]),
                    bool(ins.is_transpose),
                    str(ins.perf_mode),
                    str(ins.tile_position),
                )
                if sig == last_sig:
                    drop = True
                else:
                    last_sig = sig
            elif (
                getattr(ins, "engine", None) == mybir.EngineType.PE
                and isinstance(ins, mybir.InstMatmult)
                and ins.is_transpose
            ):
                # transpose-mode matmuls change the loaded weights
                last_sig = None
            if drop:
                pending.append(ins)
                n_drop += 1
                continue
            for p in pending:
                ins.merge_dependencies_from(p)
                removed[p.name] = ins.name
            pending = []
            keep.append(ins)
        assert not pending
        if len(keep) != len(insts):
            insts[:] = keep
    if removed:
        for bb in nc.main_func.blocks:
            for ins in bb.instructions:
                ins.remap_dependency_names(removed)
        if hasattr(nc, "inst_map"):
            for name in removed:
                nc.inst_map.pop(name, None)


def _body(nc, tc, causal, constp, dram, qT_s, vT_s, vT_t):
    with (
        tc.tile_pool(name="xTp", bufs=1) as xTp,
        tc.tile_pool(name="qstage", bufs=2) as qstagep,
        tc.tile_pool(name="wblk", bufs=1) as wblkp,
        tc.tile_pool(name="psAB", bufs=2, space="PSUM") as psABp,
    ):
        # projection-critical loads first, interleaved so (xT[k], Wv[k], Wq[k])
        # arrive together in k order
        xT_t, Wv_t, Wq_t = [], [], []
        for k in range(NTD):
            xt = xTp.tile([PT, S], BF16, tag=f"xT{k}", name=f"xT{k}")
            nc.sync.dma_start(xt[:], dram["xT"][k * PT : (k + 1) * PT, :])
            xT_t.append(xt)
            wv = wblkp.tile([PT, D], BF16, tag=f"wv{k}", name=f"wv{k}")
            nc.sync.dma_start(wv[:], dram["WvT"][k * PT : (k + 1) * PT, :])
            Wv_t.append(wv)
            wq = wblkp.tile([PT, D], BF16, tag=f"wq{k}", name=f"wq{k}")
            nc.sync.dma_start(wq[:], dram["WqT"][k * PT : (k + 1) * PT, :])
            Wq_t.append(wq)
        bq_t, bv_t = [], []
        for e in range(NTD):
            tq = constp.tile([PT, 1], FP32, tag=f"bq{e}", name=f"bq{e}")
            nc.sync.dma_start(tq[:], dram["bqc"][e * PT : (e + 1) * PT, :])
            bq_t.append(tq)
            tv = constp.tile([PT, 1], FP32, tag=f"bv{e}", name=f"bv{e}")
            nc.sync.dma_start(tv[:], dram["bvc"][e * PT : (e + 1) * PT, :])
            bv_t.append(tv)

        # ---- Phase A/B: projections vT (resident) and qT (to DRAM) ----
        for e in range(NTD):
            for which in ("v", "q"):
                W_t = Wv_t if which == "v" else Wq_t
                b_t = bv_t if which == "v" else bq_t
                ps = psABp.tile([PT, S], FP32, tag="ps", name="ps")
                for k in range(NTD):
                    for sc in range(S // 512):
                        nc.tensor.matmul(
                            ps[:, sc * 512 : (sc + 1) * 512],
                            W_t[k][:, e * PT : (e + 1) * PT],
                            xT_t[k][:, sc * 512 : (sc + 1) * 512],
                            start=(k == 0),
                            stop=(k == NTD - 1),
                        )
                if which == "v":
                    nc.scalar.activation(
                        vT_t[e][:], ps[:], ACT.Identity, bias=b_t[e][:], scale=1.0
                    )
                    nc.sync.dma_start(vT_s[e * PT : (e + 1) * PT, :], vT_t[e][:])
                else:
                    qs = qstagep.tile([PT, S], BF16, tag="qs", name="qs")
                    nc.scalar.activation(
                        qs[:], ps[:], ACT.Identity, bias=b_t[e][:], scale=1.0
                    )
                    nc.sync.dma_start(qT_s[e * PT : (e + 1) * PT, :], qs[:])

    # constants for phases C/D (emitted late so they don't delay xT/W loads)
    ident = constp.tile([PT, PT], BF16, tag="ident", name="ident")
    make_identity(nc, ident[:])
    WoT_t = []
    for d in range(NTD):
        w = constp.tile([PT, D], BF16, tag=f"wot{d}", name=f"wot{d}")
        nc.sync.dma_start(w[:], dram["WoT"][d * PT : (d + 1) * PT, :])
        WoT_t.append(w)
    bo_t = constp.tile([PT, D], FP32, tag="bo", name="bo")
    nc.sync.dma_start(bo_t[:], dram["bob"][:, :])
    ones_t = constp.tile([PT, 1], BF16, tag="ones", name="ones")
    nc.sync.dma_start(ones_t[:], dram["ones_col"][:, :])
    utri_t = constp.tile([PT, PT], BF16, tag="utri", name="utri")
    nc.sync.dma_start(utri_t[:], dram["utri"][:, :])

    # ---- Phase C: v (natural layout) = DMA-transpose(vT) via DRAM ----
    with tc.tile_pool(name="vp", bufs=1) as vp:
        v_t = [vp.tile([PT, D], BF16, tag=f"v{k}", name=f"v{k}") for k in range(NTS)]
        for k in range(NTS):
            nc.sync.dma_start(
                v_t[k][:], vT_s[:, k * PT : (k + 1) * PT], transpose=True
            )

        # ---- Phase D: attention q-blocks ----
        with (
            tc.tile_pool(name="qTbp", bufs=2) as qTbp,
            tc.tile_pool(name="eTp", bufs=1) as eTp,
            tc.tile_pool(name="ctxp", bufs=1) as ctxp,
            tc.tile_pool(name="rlp", bufs=1) as rlp,
            tc.tile_pool(name="outp", bufs=2) as outp,
            tc.tile_pool(name="psS", bufs=2, space="PSUM") as psSp,
            tc.tile_pool(name="psC", bufs=2, space="PSUM") as psCp,
            tc.tile_pool(name="psL", bufs=2, space="PSUM") as psLp,
            tc.tile_pool(name="psO", bufs=2, space="PSUM") as psOp,
        ):

            def load_qTb(c):
                q0 = c * QB
                tiles = []
                for e in range(NTD):
                    qb = qTbp.tile([PT, QB], BF16, tag=f"qTb{e}", name=f"qTb{e}")
                    nc.sync.dma_start(
                        qb[:], qT_s[e * PT : (e + 1) * PT, q0 : q0 + QB]
                    )
                    tiles.append(qb)
                return tiles

            qTb_cur = load_qTb(0)
            for c in range(NBLK):
                q0 = c * QB
                kmax = KPB * (c + 1) if causal else NTS

                # scoresT + exp -> eT tiles (ragged in the diagonal region)
                eT_t = []
                for ki in range(kmax):
                    m = ki - KPB * c  # >=0 in diagonal region
                    lo = m * PT if (causal and m > 0) else 0
                    ps = psSp.tile([PT, QB], FP32, tag="s", name="s")
                    for e in range(NTD):
                        nc.tensor.matmul(
                            ps[:, lo:QB],
                            vT_t[e][:, ki * PT : (ki + 1) * PT],
                            qTb_cur[e][:, lo:QB],
                            start=(e == 0),
                            stop=(e == NTD - 1),
                        )
                    et = eTp.tile([PT, QB], BF16, tag=f"e{ki}", name=f"e{ki}")
                    if causal and m >= 0:
                        if m > 0:
                            nc.gpsimd.memset(et[:, 0:lo], 0.0)
                        nc.scalar.activation(
                            et[:, lo:QB], ps[:, lo:QB], ACT.Exp, scale=float(SCALE)
                        )
                        nc.vector.tensor_mul(
                            et[:, m * PT : (m + 1) * PT],
                            et[:, m * PT : (m + 1) * PT],
                            utri_t[:],
                        )
                    else:
                        nc.scalar.activation(et[:], ps[:], ACT.Exp, scale=float(SCALE))
                    eT_t.append(et)

                # prefetch next block's qT while PE chews on ctx/out
                if c + 1 < NBLK:
                    qTb_next = load_qTb(c + 1)

                # ctxT[d, q-block], accumulated over k tiles (ragged on diag)
                ctx_t = []
                for d in range(NTD):
                    pc = psCp.tile([PT, QB], FP32, tag="c", name="c")
                    for ki in range(kmax):
                        m = ki - KPB * c
                        lo = m * PT if (causal and m > 0) else 0
                        nc.tensor.matmul(
                            pc[:, lo:QB],
                            v_t[ki][:, d * PT : (d + 1) * PT],
                            eT_t[ki][:, lo:QB],
                            start=(ki == 0),
                            stop=(ki == kmax - 1),
                        )
                    cx = ctxp.tile([PT, QB], BF16, tag=f"cx{d}", name=f"cx{d}")
                    nc.vector.tensor_copy(cx[:], pc[:])
                    ctx_t.append(cx)

                # softmax denominators per q sub-tile: l = eT.T @ ones
                rl_t = []
                for qt in range(KPB):
                    pl = psLp.tile([PT, 1], FP32, tag="l", name="l")
                    for ki in range(kmax):
                        nc.tensor.matmul(
                            pl[:],
                            eT_t[ki][:, qt * PT : (qt + 1) * PT],
                            ones_t[:],
                            start=(ki == 0),
                            stop=(ki == kmax - 1),
                        )
                    r = rlp.tile([PT, 1], FP32, tag=f"rl{qt}", name=f"rl{qt}")
                    nc.vector.reciprocal(r[:], pl[:])
                    rl_t.append(r)

                # out projection + normalize + bias + store
                for qt in range(KPB):
                    os_ = outp.tile([PT, D], FP32, tag="os", name="os")
                    pos = [
                        psOp.tile([PT, 512], FP32, tag="o", name="o")
                        for _ in range(D // 512)
                    ]
                    for d in range(NTD):
                        for ec in range(D // 512):
                            nc.tensor.matmul(
                                pos[ec][:],
                                ctx_t[d][:, qt * PT : (qt + 1) * PT],
                                WoT_t[d][:, ec * 512 : (ec + 1) * 512],
                                start=(d == 0),
                                stop=(d == NTD - 1),
                            )
                    for ec in range(D // 512):
                        nc.vector.tensor_scalar_mul(
                            os_[:, ec * 512 : (ec + 1) * 512], pos[ec][:], rl_t[qt][:]
                        )
                    nc.vector.tensor_add(os_[:], os_[:], bo_t[:])
                    nc.sync.dma_start(
                        dram["out"][q0 + qt * PT : q0 + (qt + 1) * PT, :], os_[:]
                    )
                if c + 1 < NBLK:
                    qTb_cur = qTb_next


_TRIL = None


def _detect_causal(mask: np.ndarray) -> bool:
    global _TRIL
    m0 = np.asarray(mask[0])
    if bool(m0[0, 1]):
        if not m0.all() or not np.asarray(mask).all():
            raise NotImplementedError("unsupported mask pattern")
        return False
    if _TRIL is None:
        _TRIL = np.tril(np.ones((S, S), dtype=bool))
    for b in range(mask.shape[0]):
        if not np.array_equal(np.asarray(mask[b]), _TRIL):
            raise NotImplementedError("unsupported mask pattern")
    return True


def kernel(x, mask, Wq, bq, Wk, bk, Wv, bv, Wo, bo):
    import ml_dtypes

    x = np.asarray(x, dtype=np.float32)
    causal = _detect_causal(np.asarray(mask))
    nc = build_nc(causal)

    WqT = np.ascontiguousarray(np.asarray(Wq, dtype=np.float32).T).astype(
        ml_dtypes.bfloat16
    )
    WvT = np.ascontiguousarray(np.asarray(Wv, dtype=np.float32).T).astype(
        ml_dtypes.bfloat16
    )
    WoT = np.ascontiguousarray(np.asarray(Wo, dtype=np.float32).T).astype(
        ml_dtypes.bfloat16
    )
    base = {
        "WqT": WqT,
        "WvT": WvT,
        "WoT": WoT,
        "bqc": np.asarray(bq, dtype=np.float32).reshape(D, 1),
        "bvc": np.asarray(bv, dtype=np.float32).reshape(D, 1),
        "bob": np.tile(np.asarray(bo, dtype=np.float32).reshape(1, D), (PT, 1)),
        "ones_col": np.ones((PT, 1), dtype=ml_dtypes.bfloat16),
        "utri": np.triu(np.ones((PT, PT), dtype=np.float32)).astype(ml_dtypes.bfloat16),
    }
    in_maps = [
        {"xT": np.ascontiguousarray(x[b].T).astype(ml_dtypes.bfloat16), **base}
        for b in range(B)
    ]
    res = run_bass_kernel_spmd(nc, in_maps, list(range(B)))
    out = np.stack([np.asarray(res.results[i]["out"]) for i in range(B)])
    return out.astype(np.float32)


if __name__ == "__main__":
    rng = np.random.default_rng(0)
    x = rng.standard_normal((B, S, D), dtype=np.float32)
    mask = np.broadcast_to(np.tril(np.ones((S, S), dtype=bool)), (B, S, S))
    mk = lambda *s: (rng.standard_normal(s, dtype=np.float32) * 0.02)
    out = kernel(
        x, mask, mk(D, D), mk(D), mk(D, D), mk(D), mk(D, D), mk(D), mk(D, D), mk(D)
    )
    print(out.shape, out.dtype)


# revision 20
# speedup vs baseline: 95.3970x; 1.0490x over previous
"""Trainium2 Bass kernel for nn_Attention_44564580663760.

Single-head "attention" (B=8, S=2048, D=1024, fp32) with the reference's
quirk reproduced: scores = q @ v^T (k projection unused), causal mask,
softmax, ctx @ v, output projection.

Sharding: data-parallel — one batch element per NeuronCore (8 cores).

Per-core dataflow (layouts chosen so the attention matrix never needs an
on-chip transpose; matmul contracts the partition dim):
    xT  [d, s]   (host-transposed input, bf16)
    qT  [e, s] = WqT.T @ xT (+bq)         -> DRAM scratch, reloaded per block
    vT  [e, s] = WvT.T @ xT (+bv)         -> resident
    v   [k, d] = PE-transpose(vT)         -> resident
    per q-block (512 cols):
      scoresT [k, q] = vT.T @ qT          (fp32 psum)
      eT = exp(scoresT/32) (bf16), causal-masked
      ctxT [d, q]: lhsT = v slices, rhs = eT
      l[q] = column sums of eT via N=1 matmuls with ones (lhsT = eT)
      out [q, e]: lhsT = ctxT slices, rhs = WoT; scaled by 1/l, + bo
"""

import sys

sys.path.insert(0, "/opt/trn_rl_repo")

import contextlib

import numpy as np

import concourse.bacc as bacc
import concourse.bass as bass
import concourse.mybir as mybir
import concourse.tile as tile
from concourse.bass_utils import run_bass_kernel_spmd
from concourse.masks import make_identity

FP32 = mybir.dt.float32
BF16 = mybir.dt.bfloat16

B, S, D = 8, 2048, 1024
PT = 128
NTS = S // PT  # 16 s-tiles
NTD = D // PT  # 8 d-tiles
QB = 512  # q-block width
NBLK = S // QB  # 4
KPB = QB // PT  # 4 k-tiles per block
SCALE = 1.0 / np.sqrt(np.float32(D))
ACT = mybir.ActivationFunctionType


def build_nc(causal: bool, reps: int = 0) -> bass.Bass:
    nc = bacc.Bacc("TRN2", target_bir_lowering=False, debug=False)
    dram = {
        "xT": nc.declare_dram_parameter("xT", [D, S], BF16, isOutput=False),
        "WqT": nc.declare_dram_parameter("WqT", [D, D], BF16, isOutput=False),
        "WvT": nc.declare_dram_parameter("WvT", [D, D], BF16, isOutput=False),
        "WoT": nc.declare_dram_parameter("WoT", [D, D], BF16, isOutput=False),
        "bqc": nc.declare_dram_parameter("bqc", [D, 1], FP32, isOutput=False),
        "bvc": nc.declare_dram_parameter("bvc", [D, 1], FP32, isOutput=False),
        "bob": nc.declare_dram_parameter("bob", [PT, D], FP32, isOutput=False),
        "ones_col": nc.declare_dram_parameter("ones_col", [PT, 1], BF16, isOutput=False),
        "utri": nc.declare_dram_parameter("utri", [PT, PT], BF16, isOutput=False),
        "out": nc.declare_dram_parameter("out", [S, D], FP32, isOutput=True),
    }

    with (
        tile.TileContext(nc) as tc,
        tc.tile_pool(name="const", bufs=1) as constp,
        tc.tile_pool(name="dramp", bufs=1, space="DRAM") as dramp,
        tc.tile_pool(name="vTp", bufs=1) as vTp,
    ):
        qT_s = dramp.tile([D, S], BF16, tag="qts", name="qts")
        vT_s = dramp.tile([D, S], BF16, tag="vts", name="vts")
        vT_t = [
            vTp.tile([PT, S], BF16, tag=f"vT{e}", name=f"vT{e}") for e in range(NTD)
        ]
        loop_ctx = tc.For_i(0, reps, 1) if reps else contextlib.nullcontext()
        with loop_ctx:
            _body(nc, tc, causal, constp, dram, qT_s, vT_s, vT_t)
    _dedup_ldweights(nc)
    nc.finalize()
    return nc


def _dedup_ldweights(nc):
    """Drop InstLdweights whose stationary operand matches the previous PE
    weight load (no intervening PE weight change) — the paired matmuls then
    reuse the already-loaded weights. Deps of a dropped LDW move to the next
    kept instruction so semaphore generation still orders correctly."""
    removed = {}
    n_drop = 0
    for bb in nc.main_func.blocks:
        insts = bb.instructions
        keep = []
        last_sig = None
        pending = []
        for ins in insts:
            drop = False
            if isinstance(ins, mybir.InstLdweights):
                sig = (
                    str(ins.ins[0]),
                    bool(ins.is_transpose),
                    str(ins.perf_mode),
                    str(ins.tile_position),
                )
                if sig == last_sig:
                    drop = True
                else:
                    last_sig = sig
            elif (
                getattr(ins, "engine", None) == mybir.EngineType.PE
                and isinstance(ins, mybir.InstMatmult)
                and ins.is_transpose
            ):
                # transpose-mode matmuls change the loaded weights
                last_sig = None
            if drop:
                pending.append(ins)
                n_drop += 1
                continue
            for p in pending:
                ins.merge_dependencies_from(p)
                removed[p.name] = ins.name
            pending = []
            keep.append(ins)
        assert not pending
        if len(keep) != len(insts):
            insts[:] = keep
    if removed:
        for bb in nc.main_func.blocks:
            for ins in bb.instructions:
                ins.remap_dependency_names(removed)
        if hasattr(nc, "inst_map"):
            for name in removed:
                nc.inst_map.pop(name, None)


def _body(nc, tc, causal, constp, dram, qT_s, vT_s, vT_t):
    with (
        tc.tile_pool(name="xTp", bufs=1) as xTp,
        tc.tile_pool(name="qstage", bufs=2) as qstagep,
        tc.tile_pool(name="wblk", bufs=1) as wblkp,
        tc.tile_pool(name="psAB", bufs=2, space="PSUM") as psABp,
    ):
        # projection-critical loads first, interleaved so (xT[k], Wv[k], Wq[k])
        # arrive together in k order
        xT_t, Wv_t, Wq_t = [], [], []
        for k in range(NTD):
            xt = xTp.tile([PT, S], BF16, tag=f"xT{k}", name=f"xT{k}")
            nc.sync.dma_start(xt[:], dram["xT"][k * PT : (k + 1) * PT, :])
            xT_t.append(xt)
            wv = wblkp.tile([PT, D], BF16, tag=f"wv{k}", name=f"wv{k}")
            nc.sync.dma_start(wv[:], dram["WvT"][k * PT : (k + 1) * PT, :])
            Wv_t.append(wv)
            wq = wblkp.tile([PT, D], BF16, tag=f"wq{k}", name=f"wq{k}")
            nc.sync.dma_start(wq[:], dram["WqT"][k * PT : (k + 1) * PT, :])
            Wq_t.append(wq)
        bq_t, bv_t = [], []
        for e in range(NTD):
            tq = constp.tile([PT, 1], FP32, tag=f"bq{e}", name=f"bq{e}")
            nc.sync.dma_start(tq[:], dram["bqc"][e * PT : (e + 1) * PT, :])
            bq_t.append(tq)
            tv = constp.tile([PT, 1], FP32, tag=f"bv{e}", name=f"bv{e}")
            nc.sync.dma_start(tv[:], dram["bvc"][e * PT : (e + 1) * PT, :])
            bv_t.append(tv)

        # ---- Phase A/B: projections vT (resident) and qT (to DRAM) ----
        for e in range(NTD):
            for which in ("v", "q"):
                W_t = Wv_t if which == "v" else Wq_t
                b_t = bv_t if which == "v" else bq_t
                ps = psABp.tile([PT, S], FP32, tag="ps", name="ps")
                for k in range(NTD):
                    for sc in range(S // 512):
                        nc.tensor.matmul(
                            ps[:, sc * 512 : (sc + 1) * 512],
                            W_t[k][:, e * PT : (e + 1) * PT],
                            xT_t[k][:, sc * 512 : (sc + 1) * 512],
                            start=(k == 0),
                            stop=(k == NTD - 1),
                        )
                if which == "v":
                    nc.scalar.activation(
                        vT_t[e][:], ps[:], ACT.Identity, bias=b_t[e][:], scale=1.0
                    )
                    nc.sync.dma_start(vT_s[e * PT : (e + 1) * PT, :], vT_t[e][:])
                else:
                    qs = qstagep.tile([PT, S], BF16, tag="qs", name="qs")
                    nc.scalar.activation(
                        qs[:], ps[:], ACT.Identity, bias=b_t[e][:], scale=1.0
                    )
                    nc.sync.dma_start(qT_s[e * PT : (e + 1) * PT, :], qs[:])

    # constants for phases C/D (emitted late so they don't delay xT/W loads)
    ident = constp.tile([PT, PT], BF16, tag="ident", name="ident")
    make_identity(nc, ident[:])
    WoT_t = []
    for d in range(NTD):
        w = constp.tile([PT, D], BF16, tag=f"wot{d}", name=f"wot{d}")
        nc.sync.dma_start(w[:], dram["WoT"][d * PT : (d + 1) * PT, :])
        WoT_t.append(w)
    bo_t = constp.tile([PT, D], FP32, tag="bo", name="bo")
    nc.sync.dma_start(bo_t[:], dram["bob"][:, :])
    ones_t = constp.tile([PT, 1], BF16, tag="ones", name="ones")
    nc.sync.dma_start(ones_t[:], dram["ones_col"][:, :])
    utri_t = constp.tile([PT, PT], BF16, tag="utri", name="utri")
    nc.sync.dma_start(utri_t[:], dram["utri"][:, :])

    # ---- Phase C: v (natural layout) = DMA-transpose(vT) via DRAM ----
    with tc.tile_pool(name="vp", bufs=1) as vp:
        v_t = [vp.tile([PT, D], BF16, tag=f"v{k}", name=f"v{k}") for k in range(NTS)]
        for k in range(NTS):
            nc.sync.dma_start(
                v_t[k][:], vT_s[:, k * PT : (k + 1) * PT], transpose=True
            )

        # ---- Phase D: attention q-blocks ----
        with (
            tc.tile_pool(name="qTbp", bufs=2) as qTbp,
            tc.tile_pool(name="eTp", bufs=1) as eTp,
            tc.tile_pool(name="ctxp", bufs=1) as ctxp,
            tc.tile_pool(name="rlp", bufs=1) as rlp,
            tc.tile_pool(name="outp", bufs=2) as outp,
            tc.tile_pool(name="psS", bufs=2, space="PSUM") as psSp,
            tc.tile_pool(name="psC", bufs=2, space="PSUM") as psCp,
            tc.tile_pool(name="psL", bufs=2, space="PSUM") as psLp,
            tc.tile_pool(name="psO", bufs=2, space="PSUM") as psOp,
        ):

            def load_qTb(c):
                q0 = c * QB
                tiles = []
                for e in range(NTD):
                    qb = qTbp.tile([PT, QB], BF16, tag=f"qTb{e}", name=f"qTb{e}")
                    nc.sync.dma_start(
                        qb[:], qT_s[e * PT : (e + 1) * PT, q0 : q0 + QB]
                    )
                    tiles.append(qb)
                return tiles

            qTb_cur = load_qTb(0)
            for c in range(NBLK):
                q0 = c * QB
                kmax = KPB * (c + 1) if causal else NTS

                # scoresT + exp -> eT tiles (ragged in the diagonal region)
                eT_t = []
                for ki in range(kmax):
                    m = ki - KPB * c  # >=0 in diagonal region
                    lo = m * PT if (causal and m > 0) else 0
                    ps = psSp.tile([PT, QB], FP32, tag="s", name="s")
                    for e in range(NTD):
                        nc.tensor.matmul(
                            ps[:, lo:QB],
                            vT_t[e][:, ki * PT : (ki + 1) * PT],
                            qTb_cur[e][:, lo:QB],
                            start=(e == 0),
                            stop=(e == NTD - 1),
                        )
                    et = eTp.tile([PT, QB], BF16, tag=f"e{ki}", name=f"e{ki}")
                    if causal and m >= 0:
                        if m > 0:
                            nc.gpsimd.memset(et[:, 0:lo], 0.0)
                        nc.scalar.activation(
                            et[:, lo:QB], ps[:, lo:QB], ACT.Exp, scale=float(SCALE)
                        )
                        nc.vector.tensor_mul(
                            et[:, m * PT : (m + 1) * PT],
                            et[:, m * PT : (m + 1) * PT],
                            utri_t[:],
                        )
                    else:
                        nc.scalar.activation(et[:], ps[:], ACT.Exp, scale=float(SCALE))
                    eT_t.append(et)

                # prefetch next block's qT while PE chews on ctx/out
                if c + 1 < NBLK:
                    qTb_next = load_qTb(c + 1)

                # ctxT[d, q-block], accumulated over k tiles (ragged on diag)
                ctx_t = []
                for d in range(NTD):
                    pc = psCp.tile([PT, QB], FP32, tag="c", name="c")
                    for ki in range(kmax):
                        m = ki - KPB * c
                        lo = m * PT if (causal and m > 0) else 0
                        nc.tensor.matmul(
                            pc[:, lo:QB],
                            v_t[ki][:, d * PT : (d + 1) * PT],
                            eT_t[ki][:, lo:QB],
                            start=(ki == 0),
                            stop=(ki == kmax - 1),
                        )
                    cx = ctxp.tile([PT, QB], BF16, tag=f"cx{d}", name=f"cx{d}")
                    nc.vector.tensor_copy(cx[:], pc[:])
                    ctx_t.append(cx)

                # softmax denominators per q sub-tile: l = eT.T @ ones
                rl_t = []
                for qt in range(KPB):
                    pl = psLp.tile([PT, 1], FP32, tag="l", name="l")
                    for ki in range(kmax):
                        nc.tensor.matmul(
                            pl[:],
                            eT_t[ki][:, qt * PT : (qt + 1) * PT],
                            ones_t[:],
                            start=(ki == 0),
                            stop=(ki == kmax - 1),
                        )
                    r = rlp.tile([PT, 1], FP32, tag=f"rl{qt}", name=f"rl{qt}")
                    nc.vector.reciprocal(r[:], pl[:])
                    rl_t.append(r)

                # out projection + normalize + bias + store
                for qt in range(KPB):
                    os_ = outp.tile([PT, D], FP32, tag="os", name="os")
                    pos = [
                        psOp.tile([PT, 512], FP32, tag="o", name="o")
                        for _ in range(D // 512)
                    ]
                    for d in range(NTD):
                        for ec in range(D // 512):
                            nc.tensor.matmul(
                                pos[ec][:],
                                ctx_t[d][:, qt * PT : (qt + 1) * PT],
                                WoT_t[d][:, ec * 512 : (ec + 1) * 512],
                                start=(d == 0),
                                stop=(d == NTD - 1),
                            )
                    for ec in range(D // 512):
                        nc.vector.tensor_scalar_mul(
                            os_[:, ec * 512 : (ec + 1) * 512], pos[ec][:], rl_t[qt][:]
                        )
                    nc.vector.tensor_add(os_[:], os_[:], bo_t[:])
                    nc.sync.dma_start(
                        dram["out"][q0 + qt * PT : q0 + (qt + 1) * PT, :], os_[:]
                    )
                if c + 1 < NBLK:
                    qTb_cur = qTb_next


_TRIL = None


def _detect_causal(mask: np.ndarray) -> bool:
    global _TRIL
    m0 = np.asarray(mask[0])
    if bool(m0[0, 1]):
        if not m0.all() or not np.asarray(mask).all():
            raise NotImplementedError("unsupported mask pattern")
        return False
    if _TRIL is None:
        _TRIL = np.tril(np.ones((S, S), dtype=bool))
    for b in range(mask.shape[0]):
        if not np.array_equal(np.asarray(mask[b]), _TRIL):
            raise NotImplementedError("unsupported mask pattern")
    return True


def kernel(x, mask, Wq, bq, Wk, bk, Wv, bv, Wo, bo):
    import ml_dtypes

    x = np.asarray(x, dtype=np.float32)
    causal = _detect_causal(np.asarray(mask))
    nc = build_nc(causal)

    WqT = np.ascontiguousarray(np.asarray(Wq, dtype=np.float32).T).astype(
        ml_dtypes.bfloat16
    )
    WvT = np.ascontiguousarray(np.asarray(Wv, dtype=np.float32).T).astype(
        ml_dtypes.bfloat16
    )
    WoT = np.ascontiguousarray(np.asarray(Wo, dtype=np.float32).T).astype(
        ml_dtypes.bfloat16
    )
    base = {
        "WqT": WqT,
        "WvT": WvT,
        "WoT": WoT,
        "bqc": np.asarray(bq, dtype=np.float32).reshape(D, 1),
        "bvc": np.asarray(bv, dtype=np.float32).reshape(D, 1),
        "bob": np.tile(np.asarray(bo, dtype=np.float32).reshape(1, D), (PT, 1)),
        "ones_col": np.ones((PT, 1), dtype=ml_dtypes.bfloat16),
        "utri": np.triu(np.ones((PT, PT), dtype=np.float32)).astype(ml_dtypes.bfloat16),
    }
    in_maps = [
        {"xT": np.ascontiguousarray(x[b].T).astype(ml_dtypes.bfloat16), **base}
        for b in range(B)
    ]
    res = run_bass_kernel_spmd(nc, in_maps, list(range(B)))
    out = np.stack([np.asarray(res.results[i]["out"]) for i in range(B)])
    return out.astype(np.float32)


if __name__ == "__main__":
    rng = np.random.default_rng(0)
    x = rng.standard_normal((B, S, D), dtype=np.float32)
    mask = np.broadcast_to(np.tril(np.ones((S, S), dtype=bool)), (B, S, S))
    mk = lambda *s: (rng.standard_normal(s, dtype=np.float32) * 0.02)
    out = kernel(
        x, mask, mk(D, D), mk(D), mk(D, D), mk(D), mk(D, D), mk(D), mk(D, D), mk(D)
    )
    print(out.shape, out.dtype)
